# revision 15
# baseline (speedup 1.0000x reference)
"""HalfEdgeConv Trainium2 kernel.

out[e] = relu(W @ concat(x[next_idx[e]], has_twin[e] ? x[twin_idx[e]] : 0) + b)

Strategy (data-parallel over half-edges, 8 cores), "two-phase batched gather":
  The HW bottleneck is the SWDGE descriptor rate; only the batched dma_gather
  extended instruction amortizes it, and its int16 indices confine one
  instruction to a 32768-row window. So:
  - Phase 1 (stage): twin rows are fetched in twin-window-sorted order by big
    dma_gather instructions (<=1024 rows, queues 0-3) and written contiguously
    to a DRAM staging table T (position-major within each chunk).
  - Phase 2 (main): edges are processed sorted by (next-window, stage-window
    of their twin row). Each chunk does two dma_gathers: next rows from x
    (single x-window) and twin rows from T (single T-window), both int16-safe.
    Per tile: two PE transposes to channel-major, two accumulating K=64
    matmuls (Wn, Wt) into PSUM, DVE bias add, ACT ReLU+bf16, contiguous store.
  - A semaphore barriers phase 2 behind phase 1's staging stores.
  - Output rows come back in slot order; the host scatters them to edge order
    while unsharding (pure row permutation).
"""
import os
import sys

sys.path.insert(0, "/opt/trn_rl_repo")

import numpy as np
import ml_dtypes
from contextlib import ExitStack

import concourse.bass as bass
import concourse.tile as tile
from concourse import bacc, mybir, bass_utils

N = 1_000_000
C = 64
NCORES = 8
P = 128
EPC = 125_000               # edges per core (8*125000 = N)
WIN = 32768                 # dma_gather index window (int16)
NWIN = (N + 1 + WIN - 1) // WIN   # 31 windows cover x rows [0, N]
PADM = 128                  # bucket padding multiple (one tile)
CHUNK_SLOTS = 1024          # max rows per dma_gather

f32 = mybir.dt.float32
bf16 = mybir.dt.bfloat16
i32 = mybir.dt.int32
i16 = mybir.dt.int16
bf16_np = ml_dtypes.bfloat16

_COMPILED = {}
LAST_EXEC_NS = None


def _try_install_ntff_shim():
    """NTFF profiling hook (trace runs only); degrade silently if absent."""
    import types, ctypes, contextlib
    if "antenv.axon_hooks" in sys.modules:
        return
    try:
        import antenv
        mod = types.ModuleType("antenv.axon_hooks")
        mod._hook = None
        mod.set_axon_ntff_profile_hook = lambda h: setattr(mod, "_hook", h)
        mod.get_axon_ntff_profile_hook = lambda: mod._hook
        sys.modules["antenv.axon_hooks"] = mod
        antenv.axon_hooks = mod
        lib = ctypes.CDLL("/opt/axon/libaxon_pjrt.so")
        if not hasattr(lib, "axon_start_nrt_profile"):
            return
        lib.axon_start_nrt_profile.argtypes = [ctypes.POINTER(ctypes.c_int64), ctypes.c_size_t]
        lib.axon_start_nrt_profile.restype = ctypes.c_int64
        lib.axon_stop_nrt_profile.argtypes = [ctypes.c_char_p]
        lib.axon_stop_nrt_profile.restype = ctypes.c_int64

        @contextlib.contextmanager
        def _hook(output_dir, device_ids):
            import jax
            jax.devices()
            if device_ids:
                ids = (ctypes.c_int64 * len(device_ids))(*device_ids)
                rc = lib.axon_start_nrt_profile(ids, len(device_ids))
            else:
                rc = lib.axon_start_nrt_profile(None, 0)
            if rc != 0:
                raise RuntimeError(f"axon_start_nrt_profile rc={rc}")
            try:
                yield
            finally:
                lib.axon_stop_nrt_profile(str(output_dir).encode())

        mod.set_axon_ntff_profile_hook(_hook)
    except Exception:
        pass


def _wrap_idx16(local):
    """dma_gather index layout: [128, n/16] with [p, s] = idx[s*16 + p%16]."""
    w = local.astype(np.int16).reshape(-1, 16).T          # [16, n/16]
    return np.tile(w, (8, 1))                              # [128, n/16]


def _chunks_of(total):
    out = []
    s = 0
    while s < total:
        n = min(CHUNK_SLOTS, total - s)
        out.append(n)
        s += n
    return out


def _build(stage_plan, main_plan, st_total, total_slots):
    """stage_plan: [(n_rows, x_window)]; main_plan: [(n_slots, x_window,
    stage_window)] — identical across cores by construction."""
    tiles = total_slots // P
    st_tiles = st_total // P
    nc = bacc.Bacc("TRN2", target_bir_lowering=False, debug=False,
                   num_swdge_queues=4)
    x_d = nc.dram_tensor("x", [N + 1, C], f32, kind="ExternalInput").ap()
    si_d = nc.dram_tensor("sidx16", [P, st_total // 16], i16,
                          kind="ExternalInput").ap()
    ni_d = nc.dram_tensor("nidx16", [P, total_slots // 16], i16,
                          kind="ExternalInput").ap()
    ri_d = nc.dram_tensor("ridx16", [P, total_slots // 16], i16,
                          kind="ExternalInput").ap()
    wn_d = nc.dram_tensor("wnt", [C, C], f32, kind="ExternalInput").ap()
    wt_d = nc.dram_tensor("wtt", [C, C], f32, kind="ExternalInput").ap()
    b_d = nc.dram_tensor("bias", [P, C], f32, kind="ExternalInput").ap()
    id_d = nc.dram_tensor("ident", [P, P], f32, kind="ExternalInput").ap()
    st_d = nc.dram_tensor("stage", [st_total, C], f32, kind="Internal").ap()
    out_d = nc.dram_tensor("out", [P, tiles * C], bf16,
                           kind="ExternalOutput").ap()

    with tile.TileContext(nc) as tc:
        with ExitStack() as ctx:
            const = ctx.enter_context(tc.tile_pool(name="const", bufs=1))
            sgp = ctx.enter_context(tc.tile_pool(name="sg", bufs=4))
            nxp = ctx.enter_context(tc.tile_pool(name="nx", bufs=3))
            twp = ctx.enter_context(tc.tile_pool(name="tw", bufs=3))
            xtp = ctx.enter_context(tc.tile_pool(name="xt", bufs=4))
            outp = ctx.enter_context(tc.tile_pool(name="outp", bufs=2))
            ptp = ctx.enter_context(tc.tile_pool(name="pt", bufs=4, space="PSUM"))
            pop = ctx.enter_context(tc.tile_pool(name="po", bufs=4, space="PSUM"))

            wn_sb = const.tile([C, C], f32)
            nc.sync.dma_start(wn_sb[:], wn_d[:])
            wt_sb = const.tile([C, C], f32)
            nc.sync.dma_start(wt_sb[:], wt_d[:])
            b_sb = const.tile([P, C], f32)
            nc.sync.dma_start(b_sb[:], b_d[:])
            id_sb = const.tile([P, P], f32)
            nc.sync.dma_start(id_sb[:], id_d[:])
            si_sb = const.tile([P, st_total // 16], i16)
            nc.sync.dma_start(si_sb[:], si_d[:])
            ni_sb = const.tile([P, total_slots // 16], i16)
            nc.sync.dma_start(ni_sb[:], ni_d[:])
            ri_sb = const.tile([P, total_slots // 16], i16)
            nc.sync.dma_start(ri_sb[:], ri_d[:])

            # ---- phase 1: stage twin rows ----
            q = 0
            pos = 0
            for n_rows, w in stage_plan:
                k = n_rows // P
                sg = sgp.tile([P, k, C], f32, tag="sg")
                nc.gpsimd.dma_gather(
                    out_ap=sg[:, :, :], in_ap=x_d[w * WIN:, :],
                    idxs_ap=si_sb[:, pos // 16:(pos + n_rows) // 16],
                    num_idxs=n_rows, num_idxs_reg=n_rows, elem_size=C,
                    queue_num=q % 4)
                q += 1
                # staged position of gathered element (p, kk) = pos + p*k + kk
                dst = st_d[pos:pos + n_rows, :].rearrange(
                    "(p k) c -> p k c", p=P, k=k)
                nc.sync.dma_start(dst, sg[:, :, :])
                pos += n_rows

            # Barrier: a 128-partition token readback queues behind every
            # staging store on all 16 HWDGE rings; a Pool-engine consumer of
            # the token then orders all phase-2 gathers after it.
            token = const.tile([P, 16], f32)
            nc.sync.dma_start(token[:], st_d[0:P, 0:16])
            tok2 = const.tile([P, 16], f32)
            nc.gpsimd.tensor_copy(tok2[:], token[:])

            # ---- phase 2: main loop ----
            slot0 = 0
            for n_slots, wn, wtw in main_plan:
                k = n_slots // P
                t0 = slot0 // P
                nx = nxp.tile([P, k, C], f32, tag="nx")
                nc.gpsimd.dma_gather(
                    out_ap=nx[:, :, :], in_ap=x_d[wn * WIN:, :],
                    idxs_ap=ni_sb[:, slot0 // 16:(slot0 + n_slots) // 16],
                    num_idxs=n_slots, num_idxs_reg=n_slots, elem_size=C,
                    queue_num=q % 4)
                q += 1
                tw = twp.tile([P, k, C], f32, tag="tw")
                nc.gpsimd.dma_gather(
                    out_ap=tw[:, :, :], in_ap=st_d[wtw * WIN:, :],
                    idxs_ap=ri_sb[:, slot0 // 16:(slot0 + n_slots) // 16],
                    num_idxs=n_slots, num_idxs_reg=n_slots, elem_size=C,
                    queue_num=q % 4)
                q += 1

                och = outp.tile([P, k * C], bf16, tag="och")
                for t in range(k):
                    pt = ptp.tile([C, 2 * P], f32, tag="pt")
                    nc.tensor.transpose(
                        out=pt[:, 0:P], in_=nx[:, t, :], identity=id_sb[:])
                    nc.tensor.transpose(
                        out=pt[:, P:2 * P], in_=tw[:, t, :], identity=id_sb[:])
                    xt = xtp.tile([C, 2 * P], f32, tag="xt")
                    nc.vector.tensor_copy(xt[:], pt[:])
                    po = pop.tile([P, C], f32, tag="po")
                    nc.tensor.matmul(
                        out=po[:], lhsT=xt[:, 0:P], rhs=wn_sb[:],
                        start=True, stop=False)
                    nc.tensor.matmul(
                        out=po[:], lhsT=xt[:, P:2 * P], rhs=wt_sb[:],
                        start=False, stop=True)
                    nc.vector.tensor_add(out=po[:], in0=po[:], in1=b_sb[:])
                    nc.scalar.activation(
                        och[:, t * C:(t + 1) * C], po[:],
                        mybir.ActivationFunctionType.Relu)
                nc.sync.dma_start(out_d[:, t0 * C:(t0 + k) * C], och[:])
                slot0 += n_slots

    nc.compile()
    return nc


def kernel(x, next_idx, twin_idx, has_twin, W, b):
    global LAST_EXEC_NS
    x = np.asarray(x, dtype=np.float32)
    next_idx = np.asarray(next_idx, dtype=np.int64)
    twin_idx = np.asarray(twin_idx, dtype=np.int64)
    has_twin = np.asarray(has_twin)
    W = np.asarray(W, dtype=np.float32)
    b = np.asarray(b, dtype=np.float32)

    trace = bool(os.environ.get("BASS_TRACE"))
    if trace:
        _try_install_ntff_shim()

    x_pad = np.concatenate([x, np.zeros((1, C), np.float32)], axis=0)
    gt_all = np.where(has_twin, twin_idx, N)

    wnT = np.ascontiguousarray(W[:, :C].T)
    wtT = np.ascontiguousarray(W[:, C:].T)
    bias = np.broadcast_to(b, (P, C)).copy()
    ident = np.eye(P, dtype=np.float32)

    # --- common plan across cores: max bucket sizes ---
    # stage buckets: twin windows
    stage_counts = np.zeros((NCORES, NWIN), np.int64)
    for c in range(NCORES):
        gt = gt_all[c * EPC:(c + 1) * EPC]
        stage_counts[c] = np.bincount(gt >> 15, minlength=NWIN)
    stage_pad = (-(-stage_counts.max(axis=0) // PADM)) * PADM
    st_total = int(stage_pad.sum())
    nstw = (st_total + WIN - 1) // WIN          # stage windows

    stage_plan = []
    for w in range(NWIN):
        for n in _chunks_of(int(stage_pad[w])):
            stage_plan.append((n, w))

    # staged position for (bucket w, rank r): needs chunk-major p-order:
    # within a chunk of n rows (k=n//P tiles): element f=kk*128+p of the
    # chunk sits at staged position chunk_base + p*k + kk.
    def staged_pos(bucket_base, ranks, chunk_sizes):
        pos = np.empty_like(ranks)
        cb = 0
        for n in chunk_sizes:
            k = n // P
            m = (ranks >= cb) & (ranks < cb + n)
            f = ranks[m] - cb
            kk = f // P
            p = f % P
            pos[m] = bucket_base + cb + p * k + kk
            cb += n
        return pos

    # main buckets: (next window, stage window of twin)
    # stage window depends only on (twin window w, rank in bucket) via the
    # common stage_pad layout -> compute per core after ranks are known.
    stage_base = np.concatenate([[0], np.cumsum(stage_pad)[:-1]])

    main_counts = np.zeros((NCORES, NWIN, 8), np.int64)
    core_data = []
    for c in range(NCORES):
        gn = next_idx[c * EPC:(c + 1) * EPC]
        gt = gt_all[c * EPC:(c + 1) * EPC]
        wt_w = gt >> 15
        # rank of each edge within its twin-window bucket (stable order)
        order_t = np.argsort(wt_w, kind="stable")
        ranks = np.empty(EPC, np.int64)
        cnt = np.bincount(wt_w, minlength=NWIN)
        csum = np.concatenate([[0], np.cumsum(cnt)[:-1]])
        ranks[order_t] = np.arange(EPC) - csum[wt_w[order_t]]
        # staged position of each edge's twin row
        pos_t = np.empty(EPC, np.int64)
        for w in range(NWIN):
            m = wt_w == w
            if not m.any():
                continue
            pos_t[m] = staged_pos(int(stage_base[w]), ranks[m],
                                  _chunks_of(int(stage_pad[w])))
        wn = gn >> 15
        wtw = pos_t >> 15
        main_counts[c] = np.zeros((NWIN, 8), np.int64)
        np.add.at(main_counts[c], (wn, wtw), 1)
        core_data.append((gn, gt, pos_t, wn, wtw))

    main_pad = (-(-main_counts.max(axis=0) // PADM)) * PADM   # [NWIN, 8]
    total_slots = int(main_pad.sum())
    tiles = total_slots // P

    main_plan = []
    for w in range(NWIN):
        for sw in range(8):
            for n in _chunks_of(int(main_pad[w, sw])):
                main_plan.append((n, w, sw))

    key = (tuple(stage_plan), tuple(main_plan), st_total, total_slots)
    if key not in _COMPILED:
        _COMPILED.clear()
        _COMPILED[key] = _build(stage_plan, main_plan, st_total, total_slots)
    nc = _COMPILED[key]

    in_maps = []
    slot_edges = []
    for c in range(NCORES):
        gn, gt, pos_t, wn, wtw = core_data[c]

        # stage indices: bucket-major by twin window, stable edge order,
        # padded with window-base dummies
        sidx = np.empty(st_total, np.int64)
        posb = 0
        order_t = np.argsort(gt >> 15, kind="stable")
        gts = gt[order_t]
        wts = gts >> 15
        off = 0
        for w in range(NWIN):
            cw = int(np.count_nonzero(wts == w))
            pw = int(stage_pad[w])
            sidx[posb:posb + cw] = gts[off:off + cw] - (w << 15)
            sidx[posb + cw:posb + pw] = 0
            posb += pw
            off += cw
        si16 = _wrap_idx16(sidx)

        # main slots: bucket-major by (wn, wtw), stable edge order
        bucket = wn * 8 + wtw
        order_m = np.argsort(bucket, kind="stable")
        slot_gn = np.empty(total_slots, np.int64)
        slot_pt = np.empty(total_slots, np.int64)
        slot_edge = np.full(total_slots, -1, np.int64)
        cntb = np.bincount(bucket, minlength=NWIN * 8).reshape(NWIN, 8)
        posm = 0
        offm = 0
        for w in range(NWIN):
            for sw in range(8):
                cw = int(cntb[w, sw])
                pw = int(main_pad[w, sw])
                slot_gn[posm:posm + cw] = gn[order_m[offm:offm + cw]]
                slot_gn[posm + cw:posm + pw] = w << 15
                slot_pt[posm:posm + cw] = pos_t[order_m[offm:offm + cw]]
                slot_pt[posm + cw:posm + pw] = sw << 15
                slot_edge[posm:posm + cw] = order_m[offm:offm + cw]
                posm += pw
                offm += cw
        slot_edges.append(slot_edge)

        ni16 = _wrap_idx16(slot_gn - ((slot_gn >> 15) << 15))
        ri16 = _wrap_idx16(slot_pt - ((slot_pt >> 15) << 15))

        in_maps.append({"x": x_pad, "sidx16": si16, "nidx16": ni16,
                        "ridx16": ri16, "wnt": wnT, "wtt": wtT,
                        "bias": bias, "ident": ident})

    res = bass_utils.run_bass_kernel_spmd(
        nc, in_maps, core_ids=list(range(NCORES)), trace=trace)
    LAST_EXEC_NS = res.exec_time_ns

    out = np.empty((N, C), np.float32)
    for c in range(NCORES):
        o = np.asarray(res.results[c]["out"]).reshape(P, tiles, C)
        rows = o.transpose(1, 0, 2).reshape(total_slots, C).astype(np.float32)
        se = slot_edges[c]
        valid = se >= 0
        out[c * EPC + se[valid]] = rows[valid]
    return out


# revision 16
# speedup vs baseline: 1.2419x; 1.2419x over previous
"""HalfEdgeConv Trainium2 kernel.

out[e] = relu(W @ concat(x[next_idx[e]], has_twin[e] ? x[twin_idx[e]] : 0) + b)

Strategy (data-parallel over half-edges, 8 cores), "hybrid gather":
  The HW bottleneck is DMA descriptor rate (~one 256B random row per ~10ns
  per SWDGE queue). To use more than one queue:
  - Each core's edges are processed SORTED by next_idx window (32768-row
    windows), so the next-half rows can be fetched by big batched dma_gather
    instructions (<=1024 rows each, int16 in-window indices) on queues 1-3.
  - The twin-half rows are order-agnostic and fetched by classic [128,1]
    indirect DMAs (queue 0), one per 128-edge tile.
  - Per tile: PE transposes next/twin [128,64] to channel-major PSUM, DVE
    copies to SBUF, two accumulating K=64 matmuls (Wn, Wt) into PSUM, DVE
    adds bias, ACT applies ReLU + bf16 cast; contiguous chunked stores.
  - Output rows come back in sorted-slot order; the host scatters them back
    to edge order while unsharding (pure row permutation).
"""
import os
import sys

sys.path.insert(0, "/opt/trn_rl_repo")

import numpy as np
import ml_dtypes
from contextlib import ExitStack

import concourse.bass as bass
import concourse.tile as tile
from concourse import bacc, mybir, bass_utils

N = 1_000_000
C = 64
NCORES = 8
P = 128
EPC = 125_000               # edges per core (8*125000 = N)
WIN = 32768                 # dma_gather index window (int16)
NWIN = (N + 1 + WIN - 1) // WIN   # 31 windows cover rows [0, N]
PADM = 256                  # per-bucket slot padding multiple (even tiles)
CHUNK_SLOTS = 1024          # max slots per dma_gather

f32 = mybir.dt.float32
bf16 = mybir.dt.bfloat16
i32 = mybir.dt.int32
i16 = mybir.dt.int16
bf16_np = ml_dtypes.bfloat16

_COMPILED = {}
LAST_EXEC_NS = None


def _try_install_ntff_shim():
    """NTFF profiling hook (trace runs only); degrade silently if absent."""
    import types, ctypes, contextlib
    if "antenv.axon_hooks" in sys.modules:
        return
    try:
        import antenv
        mod = types.ModuleType("antenv.axon_hooks")
        mod._hook = None
        mod.set_axon_ntff_profile_hook = lambda h: setattr(mod, "_hook", h)
        mod.get_axon_ntff_profile_hook = lambda: mod._hook
        sys.modules["antenv.axon_hooks"] = mod
        antenv.axon_hooks = mod
        lib = ctypes.CDLL("/opt/axon/libaxon_pjrt.so")
        if not hasattr(lib, "axon_start_nrt_profile"):
            return
        lib.axon_start_nrt_profile.argtypes = [ctypes.POINTER(ctypes.c_int64), ctypes.c_size_t]
        lib.axon_start_nrt_profile.restype = ctypes.c_int64
        lib.axon_stop_nrt_profile.argtypes = [ctypes.c_char_p]
        lib.axon_stop_nrt_profile.restype = ctypes.c_int64

        @contextlib.contextmanager
        def _hook(output_dir, device_ids):
            import jax
            jax.devices()
            if device_ids:
                ids = (ctypes.c_int64 * len(device_ids))(*device_ids)
                rc = lib.axon_start_nrt_profile(ids, len(device_ids))
            else:
                rc = lib.axon_start_nrt_profile(None, 0)
            if rc != 0:
                raise RuntimeError(f"axon_start_nrt_profile rc={rc}")
            try:
                yield
            finally:
                lib.axon_stop_nrt_profile(str(output_dir).encode())

        mod.set_axon_ntff_profile_hook(_hook)
    except Exception:
        pass


def _wrap_idx16(local):
    """dma_gather index layout: [128, n/16] with [p, s] = idx[s*16 + p%16]."""
    w = local.astype(np.int16).reshape(-1, 16).T          # [16, n/16]
    return np.tile(w, (8, 1))                              # [128, n/16]


def _build(chunk_plan, total_slots):
    """chunk_plan: list of (n_slots, window) per chunk — identical across
    cores by construction of the build key."""
    tiles = total_slots // P
    nc = bacc.Bacc("TRN2", target_bir_lowering=False, debug=False,
                   num_swdge_queues=4)
    x_d = nc.dram_tensor("x", [N + 1, C], f32, kind="ExternalInput").ap()
    ni_d = nc.dram_tensor("nidx16", [P, total_slots // 16], i16,
                          kind="ExternalInput").ap()
    ti_d = nc.dram_tensor("tidx", [P, tiles], i32, kind="ExternalInput").ap()
    wn_d = nc.dram_tensor("wnt", [P, C], f32, kind="ExternalInput").ap()
    wt_d = nc.dram_tensor("wtt", [P, C], f32, kind="ExternalInput").ap()
    b_d = nc.dram_tensor("bias", [P, 2 * C], f32, kind="ExternalInput").ap()
    id_d = nc.dram_tensor("ident", [P, P], f32, kind="ExternalInput").ap()
    out_d = nc.dram_tensor("out", [P, tiles * C], bf16, kind="ExternalOutput").ap()

    with tile.TileContext(nc) as tc:
        with ExitStack() as ctx:
            const = ctx.enter_context(tc.tile_pool(name="const", bufs=1))
            nxp = ctx.enter_context(tc.tile_pool(name="nx", bufs=3))
            twp = ctx.enter_context(tc.tile_pool(name="tw", bufs=3))
            xtp = ctx.enter_context(tc.tile_pool(name="xt", bufs=4))
            outp = ctx.enter_context(tc.tile_pool(name="outp", bufs=2))
            ptp = ctx.enter_context(tc.tile_pool(name="pt", bufs=4, space="PSUM"))
            pop = ctx.enter_context(tc.tile_pool(name="po", bufs=4, space="PSUM"))

            wn_sb = const.tile([P, C], f32)
            nc.sync.dma_start(wn_sb[:], wn_d[:])
            wt_sb = const.tile([P, C], f32)
            nc.sync.dma_start(wt_sb[:], wt_d[:])
            b_sb = const.tile([P, 2 * C], f32)
            nc.sync.dma_start(b_sb[:], b_d[:])
            id_sb = const.tile([P, P], f32)
            nc.sync.dma_start(id_sb[:], id_d[:])
            ni_sb = const.tile([P, total_slots // 16], i16)
            nc.sync.dma_start(ni_sb[:], ni_d[:])
            ti_sb = const.tile([P, tiles], i32)
            nc.sync.dma_start(ti_sb[:], ti_d[:])

            slot0 = 0
            gq = 0
            for ci, (n_slots, w) in enumerate(chunk_plan):
                k = n_slots // P                       # tiles in chunk (even)
                t0 = slot0 // P
                # next rows: one batched dma_gather on queues 1-3
                nx = nxp.tile([P, k, C], f32, tag="nx")
                nc.gpsimd.dma_gather(
                    out_ap=nx[:, :, :], in_ap=x_d[w * WIN:, :],
                    idxs_ap=ni_sb[:, slot0 // 16:(slot0 + n_slots) // 16],
                    num_idxs=n_slots, num_idxs_reg=n_slots, elem_size=C,
                    queue_num=1 + (gq % 3))
                gq += 1
                # twin rows: [128,1] indirect per tile on queue 0
                tw = twp.tile([P, k * C], f32, tag="tw")
                for t in range(k):
                    nc.gpsimd.indirect_dma_start(
                        out=tw[:, t * C:(t + 1) * C], out_offset=None,
                        in_=x_d[:],
                        in_offset=bass.IndirectOffsetOnAxis(
                            ap=ti_sb[:, t0 + t:t0 + t + 1], axis=0))

                och = outp.tile([P, k * C], bf16, tag="och")
                for t2 in range(k // 2):               # two tiles per group
                    po = pop.tile([P, 2 * C], f32, tag="po")
                    for half in range(2):              # tiles 2*t2, 2*t2+1
                        t = t2 * 2 + half
                        # channel-major x for this tile: [64ch, 128 slots]
                        pt = ptp.tile([C, 2 * P], f32, tag="pt")
                        nc.tensor.transpose(
                            out=pt[:, 0:P], in_=nx[:, t, :],
                            identity=id_sb[:])
                        nc.tensor.transpose(
                            out=pt[:, P:2 * P],
                            in_=tw[:, t * C:(t + 1) * C],
                            identity=id_sb[:])
                        xt = xtp.tile([C, 2 * P], f32, tag="xt")
                        nc.vector.tensor_copy(xt[:], pt[:])
                        nc.tensor.matmul(
                            out=po[:, half * C:(half + 1) * C],
                            lhsT=xt[:, 0:P], rhs=wn_sb[0:C, :],
                            start=True, stop=False)
                        nc.tensor.matmul(
                            out=po[:, half * C:(half + 1) * C],
                            lhsT=xt[:, P:2 * P], rhs=wt_sb[0:C, :],
                            start=False, stop=True)
                    nc.vector.tensor_add(out=po[:], in0=po[:], in1=b_sb[:])
                    nc.scalar.activation(
                        och[:, t2 * 2 * C:(t2 + 1) * 2 * C], po[:],
                        mybir.ActivationFunctionType.Relu)
                nc.sync.dma_start(out_d[:, t0 * C:(t0 + k) * C], och[:])
                slot0 += n_slots

    nc.compile()
    return nc


def kernel(x, next_idx, twin_idx, has_twin, W, b):
    global LAST_EXEC_NS
    x = np.asarray(x, dtype=np.float32)
    next_idx = np.asarray(next_idx, dtype=np.int64)
    twin_idx = np.asarray(twin_idx, dtype=np.int64)
    has_twin = np.asarray(has_twin)
    W = np.asarray(W, dtype=np.float32)
    b = np.asarray(b, dtype=np.float32)

    trace = bool(os.environ.get("BASS_TRACE"))
    if trace:
        _try_install_ntff_shim()

    x_pad = np.concatenate([x, np.zeros((1, C), np.float32)], axis=0)
    gt_all = np.where(has_twin, twin_idx, N)

    wnT = np.tile(np.ascontiguousarray(W[:, :C].T), (2, 1))   # [128, 64]
    wtT = np.tile(np.ascontiguousarray(W[:, C:].T), (2, 1))   # [128, 64]
    bias = np.broadcast_to(np.tile(b, 2), (P, 2 * C)).copy()
    ident = np.eye(P, dtype=np.float32)

    # All cores share one program, so bucket sizes are padded to the max
    # across cores (the SPMD program bakes in the per-window chunk structure;
    # only the index tensors differ per core).
    counts = np.zeros((NCORES, NWIN), np.int64)
    for c in range(NCORES):
        gn = next_idx[c * EPC:(c + 1) * EPC]
        counts[c] = np.bincount(gn >> 15, minlength=NWIN)
    cmax = counts.max(axis=0)
    padded = (-(-cmax // PADM)) * PADM          # common per-window slot count

    chunk_plan = []
    for w in range(NWIN):
        pw = int(padded[w])
        s = 0
        while s < pw:
            n = min(CHUNK_SLOTS, pw - s)
            chunk_plan.append((n, w))
            s += n
    total_slots = int(padded.sum())
    tiles = total_slots // P

    in_maps = []
    slot_edges = []
    for c in range(NCORES):
        sl = slice(c * EPC, (c + 1) * EPC)
        gn = next_idx[sl]
        gt = gt_all[sl]
        wn = gn >> 15
        order = np.argsort(wn, kind="stable")
        gn_s = gn[order]
        gt_s = gt[order]
        cnt = counts[c]

        slot_gn = np.empty(total_slots, np.int64)
        slot_gt = np.full(total_slots, N, np.int64)
        slot_edge = np.full(total_slots, -1, np.int64)
        pos = 0
        off = 0
        for w in range(NWIN):
            cw = int(cnt[w])
            pw = int(padded[w])
            slot_gn[pos:pos + cw] = gn_s[off:off + cw]
            slot_gn[pos + cw:pos + pw] = w << 15
            slot_gt[pos:pos + cw] = gt_s[off:off + cw]
            slot_edge[pos:pos + cw] = order[off:off + cw]
            pos += pw
            off += cw
        slot_edges.append(slot_edge)

        wbase = np.repeat(np.arange(NWIN, dtype=np.int64) << 15, padded)
        local = slot_gn - wbase
        ni16 = _wrap_idx16(local)                        # [128, total/16]
        # twin: [p, tile] = slot_gt[tile*128 + p]
        ti32 = np.ascontiguousarray(
            slot_gt.reshape(tiles, P).T.astype(np.int32))
        in_maps.append({"x": x_pad, "nidx16": ni16, "tidx": ti32,
                        "wnt": wnT, "wtt": wtT, "bias": bias, "ident": ident})

    key = (tuple(chunk_plan), total_slots)
    if key not in _COMPILED:
        _COMPILED.clear()
        _COMPILED[key] = _build(chunk_plan, total_slots)
    nc = _COMPILED[key]

    res = bass_utils.run_bass_kernel_spmd(
        nc, in_maps, core_ids=list(range(NCORES)), trace=trace)
    LAST_EXEC_NS = res.exec_time_ns

    out = np.empty((N, C), np.float32)
    for c in range(NCORES):
        o = np.asarray(res.results[c]["out"]).reshape(P, tiles, C)
        rows = o.transpose(1, 0, 2).reshape(total_slots, C).astype(np.float32)
        se = slot_edges[c]
        valid = se >= 0
        out[c * EPC + se[valid]] = rows[valid]
    return out


# revision 18
# speedup vs baseline: 1.2538x; 1.0096x over previous
"""HalfEdgeConv Trainium2 kernel.

out[e] = relu(W @ concat(x[next_idx[e]], has_twin[e] ? x[twin_idx[e]] : 0) + b)

Strategy (data-parallel over half-edges, 8 cores), "hybrid gather":
  The HW bottleneck is DMA descriptor rate (~one 256B random row per ~10ns
  per SWDGE queue). To use more than one queue:
  - Each core's edges are processed SORTED by next_idx window (32768-row
    windows), so the next-half rows can be fetched by big batched dma_gather
    instructions (<=1024 rows each, int16 in-window indices) on queues 1-3.
  - The twin-half rows are order-agnostic and fetched by classic [128,1]
    indirect DMAs (queue 0), one per 128-edge tile.
  - Per tile: PE transposes next/twin [128,64] to channel-major PSUM, DVE
    copies to SBUF, two accumulating K=64 matmuls (Wn, Wt) into PSUM, DVE
    adds bias, ACT applies ReLU + bf16 cast; contiguous chunked stores.
  - Output rows come back in sorted-slot order; the host scatters them back
    to edge order while unsharding (pure row permutation).
"""
import os
import sys

sys.path.insert(0, "/opt/trn_rl_repo")

import numpy as np
import ml_dtypes
from contextlib import ExitStack

import concourse.bass as bass
import concourse.tile as tile
from concourse import bacc, mybir, bass_utils

N = 1_000_000
C = 64
NCORES = 8
P = 128
EPC = 125_000               # edges per core (8*125000 = N)
WIN = 32768                 # dma_gather index window (int16)
NWIN = (N + 1 + WIN - 1) // WIN   # 31 windows cover rows [0, N]
PADM = 256                  # per-bucket slot padding multiple (even tiles)
CHUNK_SLOTS = 1024          # max slots per dma_gather

f32 = mybir.dt.float32
bf16 = mybir.dt.bfloat16
i32 = mybir.dt.int32
i16 = mybir.dt.int16
bf16_np = ml_dtypes.bfloat16

_COMPILED = {}
LAST_EXEC_NS = None


def _try_install_ntff_shim():
    """NTFF profiling hook (trace runs only); degrade silently if absent."""
    import types, ctypes, contextlib
    if "antenv.axon_hooks" in sys.modules:
        return
    try:
        import antenv
        mod = types.ModuleType("antenv.axon_hooks")
        mod._hook = None
        mod.set_axon_ntff_profile_hook = lambda h: setattr(mod, "_hook", h)
        mod.get_axon_ntff_profile_hook = lambda: mod._hook
        sys.modules["antenv.axon_hooks"] = mod
        antenv.axon_hooks = mod
        lib = ctypes.CDLL("/opt/axon/libaxon_pjrt.so")
        if not hasattr(lib, "axon_start_nrt_profile"):
            return
        lib.axon_start_nrt_profile.argtypes = [ctypes.POINTER(ctypes.c_int64), ctypes.c_size_t]
        lib.axon_start_nrt_profile.restype = ctypes.c_int64
        lib.axon_stop_nrt_profile.argtypes = [ctypes.c_char_p]
        lib.axon_stop_nrt_profile.restype = ctypes.c_int64

        @contextlib.contextmanager
        def _hook(output_dir, device_ids):
            import jax
            jax.devices()
            if device_ids:
                ids = (ctypes.c_int64 * len(device_ids))(*device_ids)
                rc = lib.axon_start_nrt_profile(ids, len(device_ids))
            else:
                rc = lib.axon_start_nrt_profile(None, 0)
            if rc != 0:
                raise RuntimeError(f"axon_start_nrt_profile rc={rc}")
            try:
                yield
            finally:
                lib.axon_stop_nrt_profile(str(output_dir).encode())

        mod.set_axon_ntff_profile_hook(_hook)
    except Exception:
        pass


def _wrap_idx16(local):
    """dma_gather index layout: [128, n/16] with [p, s] = idx[s*16 + p%16]."""
    w = local.astype(np.int16).reshape(-1, 16).T          # [16, n/16]
    return np.tile(w, (8, 1))                              # [128, n/16]


def _build(chunk_plan, total_slots):
    """chunk_plan: list of (n_slots, window) per chunk — identical across
    cores by construction of the build key."""
    tiles = total_slots // P
    nc = bacc.Bacc("TRN2", target_bir_lowering=False, debug=False,
                   num_swdge_queues=4)
    x_d = nc.dram_tensor("x", [N + 1, C], f32, kind="ExternalInput").ap()
    ni_d = nc.dram_tensor("nidx16", [P, total_slots // 16], i16,
                          kind="ExternalInput").ap()
    ti_d = nc.dram_tensor("tidx", [P, tiles], i32, kind="ExternalInput").ap()
    wn_d = nc.dram_tensor("wnt", [P, C], f32, kind="ExternalInput").ap()
    wt_d = nc.dram_tensor("wtt", [P, C], f32, kind="ExternalInput").ap()
    b_d = nc.dram_tensor("bias", [P, 2 * C], f32, kind="ExternalInput").ap()
    id_d = nc.dram_tensor("ident", [P, P], f32, kind="ExternalInput").ap()
    out_d = nc.dram_tensor("out", [P, tiles * C], bf16, kind="ExternalOutput").ap()

    with tile.TileContext(nc) as tc:
        with ExitStack() as ctx:
            const = ctx.enter_context(tc.tile_pool(name="const", bufs=1))
            nxp = ctx.enter_context(tc.tile_pool(name="nx", bufs=6))
            twp = ctx.enter_context(tc.tile_pool(name="tw", bufs=6))
            xtp = ctx.enter_context(tc.tile_pool(name="xt", bufs=6))
            outp = ctx.enter_context(tc.tile_pool(name="outp", bufs=3))
            ptp = ctx.enter_context(tc.tile_pool(name="pt", bufs=4, space="PSUM"))
            pop = ctx.enter_context(tc.tile_pool(name="po", bufs=4, space="PSUM"))

            wn_sb = const.tile([P, C], f32)
            nc.sync.dma_start(wn_sb[:], wn_d[:])
            wt_sb = const.tile([P, C], f32)
            nc.sync.dma_start(wt_sb[:], wt_d[:])
            b_sb = const.tile([P, 2 * C], f32)
            nc.sync.dma_start(b_sb[:], b_d[:])
            id_sb = const.tile([P, P], f32)
            nc.sync.dma_start(id_sb[:], id_d[:])
            ni_sb = const.tile([P, total_slots // 16], i16)
            nc.sync.dma_start(ni_sb[:], ni_d[:])
            ti_sb = const.tile([P, tiles], i32)
            nc.sync.dma_start(ti_sb[:], ti_d[:])

            slot0 = 0
            gq = 0
            for ci, (n_slots, w) in enumerate(chunk_plan):
                k = n_slots // P                       # tiles in chunk (even)
                t0 = slot0 // P
                # next rows: one batched dma_gather on queues 1-3
                nx = nxp.tile([P, k, C], f32, tag="nx")
                nc.gpsimd.dma_gather(
                    out_ap=nx[:, :, :], in_ap=x_d[w * WIN:, :],
                    idxs_ap=ni_sb[:, slot0 // 16:(slot0 + n_slots) // 16],
                    num_idxs=n_slots, num_idxs_reg=n_slots, elem_size=C,
                    queue_num=1 + (gq % 3))
                gq += 1
                # twin rows: [128,1] indirect per tile on queue 0
                tw = twp.tile([P, k * C], f32, tag="tw")
                for t in range(k):
                    nc.gpsimd.indirect_dma_start(
                        out=tw[:, t * C:(t + 1) * C], out_offset=None,
                        in_=x_d[:],
                        in_offset=bass.IndirectOffsetOnAxis(
                            ap=ti_sb[:, t0 + t:t0 + t + 1], axis=0))

                och = outp.tile([P, k * C], bf16, tag="och")
                for t2 in range(k // 2):               # two tiles per group
                    po = pop.tile([P, 2 * C], f32, tag="po")
                    for half in range(2):              # tiles 2*t2, 2*t2+1
                        t = t2 * 2 + half
                        # channel-major x for this tile: [64ch, 128 slots]
                        pt = ptp.tile([C, 2 * P], f32, tag="pt")
                        nc.tensor.transpose(
                            out=pt[:, 0:P], in_=nx[:, t, :],
                            identity=id_sb[:])
                        nc.tensor.transpose(
                            out=pt[:, P:2 * P],
                            in_=tw[:, t * C:(t + 1) * C],
                            identity=id_sb[:])
                        xt = xtp.tile([C, 2 * P], f32, tag="xt")
                        nc.vector.tensor_copy(xt[:], pt[:])
                        nc.tensor.matmul(
                            out=po[:, half * C:(half + 1) * C],
                            lhsT=xt[:, 0:P], rhs=wn_sb[0:C, :],
                            start=True, stop=False)
                        nc.tensor.matmul(
                            out=po[:, half * C:(half + 1) * C],
                            lhsT=xt[:, P:2 * P], rhs=wt_sb[0:C, :],
                            start=False, stop=True)
                    nc.vector.tensor_add(out=po[:], in0=po[:], in1=b_sb[:])
                    nc.scalar.activation(
                        och[:, t2 * 2 * C:(t2 + 1) * 2 * C], po[:],
                        mybir.ActivationFunctionType.Relu)
                nc.sync.dma_start(out_d[:, t0 * C:(t0 + k) * C], och[:])
                slot0 += n_slots

    nc.compile()
    return nc


def kernel(x, next_idx, twin_idx, has_twin, W, b):
    global LAST_EXEC_NS
    x = np.asarray(x, dtype=np.float32)
    next_idx = np.asarray(next_idx, dtype=np.int64)
    twin_idx = np.asarray(twin_idx, dtype=np.int64)
    has_twin = np.asarray(has_twin)
    W = np.asarray(W, dtype=np.float32)
    b = np.asarray(b, dtype=np.float32)

    trace = bool(os.environ.get("BASS_TRACE"))
    if trace:
        _try_install_ntff_shim()

    x_pad = np.concatenate([x, np.zeros((1, C), np.float32)], axis=0)
    gt_all = np.where(has_twin, twin_idx, N)

    wnT = np.tile(np.ascontiguousarray(W[:, :C].T), (2, 1))   # [128, 64]
    wtT = np.tile(np.ascontiguousarray(W[:, C:].T), (2, 1))   # [128, 64]
    bias = np.broadcast_to(np.tile(b, 2), (P, 2 * C)).copy()
    ident = np.eye(P, dtype=np.float32)

    # All cores share one program, so bucket sizes are padded to the max
    # across cores (the SPMD program bakes in the per-window chunk structure;
    # only the index tensors differ per core).
    counts = np.zeros((NCORES, NWIN), np.int64)
    for c in range(NCORES):
        gn = next_idx[c * EPC:(c + 1) * EPC]
        counts[c] = np.bincount(gn >> 15, minlength=NWIN)
    cmax = counts.max(axis=0)
    padded = (-(-cmax // PADM)) * PADM          # common per-window slot count

    chunk_plan = []
    for w in range(NWIN):
        pw = int(padded[w])
        s = 0
        while s < pw:
            n = min(CHUNK_SLOTS, pw - s)
            chunk_plan.append((n, w))
            s += n
    total_slots = int(padded.sum())
    tiles = total_slots // P

    in_maps = []
    slot_edges = []
    for c in range(NCORES):
        sl = slice(c * EPC, (c + 1) * EPC)
        gn = next_idx[sl]
        gt = gt_all[sl]
        wn = gn >> 15
        order = np.argsort(wn, kind="stable")
        gn_s = gn[order]
        gt_s = gt[order]
        cnt = counts[c]

        slot_gn = np.empty(total_slots, np.int64)
        slot_gt = np.full(total_slots, N, np.int64)
        slot_edge = np.full(total_slots, -1, np.int64)
        pos = 0
        off = 0
        for w in range(NWIN):
            cw = int(cnt[w])
            pw = int(padded[w])
            slot_gn[pos:pos + cw] = gn_s[off:off + cw]
            slot_gn[pos + cw:pos + pw] = w << 15
            slot_gt[pos:pos + cw] = gt_s[off:off + cw]
            slot_edge[pos:pos + cw] = order[off:off + cw]
            pos += pw
            off += cw
        slot_edges.append(slot_edge)

        wbase = np.repeat(np.arange(NWIN, dtype=np.int64) << 15, padded)
        local = slot_gn - wbase
        ni16 = _wrap_idx16(local)                        # [128, total/16]
        # twin: [p, tile] = slot_gt[tile*128 + p]
        ti32 = np.ascontiguousarray(
            slot_gt.reshape(tiles, P).T.astype(np.int32))
        in_maps.append({"x": x_pad, "nidx16": ni16, "tidx": ti32,
                        "wnt": wnT, "wtt": wtT, "bias": bias, "ident": ident})

    key = (tuple(chunk_plan), total_slots)
    if key not in _COMPILED:
        _COMPILED.clear()
        _COMPILED[key] = _build(chunk_plan, total_slots)
    nc = _COMPILED[key]

    res = bass_utils.run_bass_kernel_spmd(
        nc, in_maps, core_ids=list(range(NCORES)), trace=trace)
    LAST_EXEC_NS = res.exec_time_ns

    out = np.empty((N, C), np.float32)
    for c in range(NCORES):
        o = np.asarray(res.results[c]["out"]).reshape(P, tiles, C)
        rows = o.transpose(1, 0, 2).reshape(total_slots, C).astype(np.float32)
        se = slot_edges[c]
        valid = se >= 0
        out[c * EPC + se[valid]] = rows[valid]
    return out
